# revision 1
# baseline (speedup 1.0000x reference)
"""Multi-head attention (B=4, S=2048, HID=1024, H=16, D=64) on 8 trn2 cores.

Sharding: batch x query-sequence (4 x 2), zero collectives. Each core owns one
(batch, seq-half): it computes K/V projections for the full sequence of its
batch (duplicated across the seq pair), Q projection for its 1024 queries,
attention, and the o-projection rows for its queries. Host concatenates.

Per-core dataflow (all matmuls in float32r: fp22 multiply, fp32 accumulate):
  - K.T, Q.T computed in [feature, token] layout (feeds logits directly)
  - V computed in natural [token, feature] layout with a ones column per head
    (V' = [v_h | 1]), so the AV matmul also yields the softmax denominator
  - logits computed transposed: L.T[k, q] = K_h.T.T @ Q_h.T, two heads packed
    via PE row-tiling (d=64 each at partition 0/64)
  - softmax without max-subtraction (logits ~N(0,1), exp is safe in fp32):
    P.T = exp(L.T / 8) on the scalar engine, one [128,1024] op per (kt, pair)
  - values'.T[d+1, q] accumulated over k-tiles; row 64 is the denominator
  - normalize: denominator row -> PE outer-product broadcast -> reciprocal
    -> multiply; head pairs assembled at partitions 0-63/64-127 for o_proj
  - o_proj accumulates over 8 head-pair tiles into [tok, of] and streams out
"""
import sys
sys.path.insert(0, "/opt/trn_rl_repo")
import numpy as np

import concourse.bass as bass
import concourse.mybir as mybir
import concourse.tile as tile
from concourse import bacc
from concourse.bass_utils import run_bass_kernel_spmd

F32 = mybir.dt.float32
F32R = mybir.dt.float32r
EXP = mybir.ActivationFunctionType.Exp

B, S, HID, H, D = 4, 2048, 1024, 16, 64
SQ = S // 2            # queries per core
HT = HID // 128        # 8 hid tiles
KT = S // 128          # 16 key-token tiles
TB = S // 512          # 4 token blocks (proj)
QB = SQ // 512         # 2 query blocks
NP = H // 2            # 8 head pairs
N_CORES = 8


def build_nc(n_iter: int = 1, phases=("v", "q", "pair", "o")):
    nc = bacc.Bacc(None, target_bir_lowering=False)

    xt = nc.dram_tensor("xt", [HID, S], F32R, kind="ExternalInput")
    xtq = nc.dram_tensor("xtq", [HID, SQ], F32R, kind="ExternalInput")
    wq = nc.dram_tensor("wq", [NP * HID, 128], F32R, kind="ExternalInput")
    wk = nc.dram_tensor("wk", [NP * HID, 128], F32R, kind="ExternalInput")
    wv = nc.dram_tensor("wv", [HID, HID], F32R, kind="ExternalInput")
    wo = nc.dram_tensor("wo", [HID, HID], F32R, kind="ExternalInput")
    ones16 = nc.dram_tensor("ones16", [128, 16], F32R, kind="ExternalInput")
    cone = nc.dram_tensor("cone", [1, 64], F32R, kind="ExternalInput")
    o = nc.dram_tensor("o", [SQ, HID], F32, kind="ExternalOutput")

    with tile.TileContext(nc) as tc:
        def body():
            with (
                tc.tile_pool(name="const", bufs=1) as constp,
                tc.tile_pool(name="vdramp", bufs=1, space="DRAM") as vdramp,
                tc.tile_pool(name="vnp", bufs=1) as vnp,
            ):
                ones_sb = constp.tile([1, 64], F32R)
                on16_sb = constp.tile([128, 16], F32R)
                nc.sync.dma_start(ones_sb[:], cone[:])
                nc.sync.dma_start(on16_sb[:], ones16[:])
                vdram = vdramp.tile([KT * 128, H * 65], F32R)
                vn_all = vnp.tile([128, NP * SQ], F32R)

                with (
                    tc.tile_pool(name="xtp", bufs=1) as xtp,
                    tc.tile_pool(name="qtp", bufs=1) as qtp,
                ):
                    xt_sb = [xtp.tile([128, S], F32R, name=f"xt{t}") for t in range(HT)]
                    for t in range(HT):
                        nc.sync.dma_start(xt_sb[t][:], xt[128 * t:128 * (t + 1), :])
                    qt_sb = [qtp.tile([128, SQ], F32R, name=f"qt{p}") for p in range(NP)]

                    with tc.tile_pool(name="psA", bufs=4, space="PSUM") as psA:
                        # ---- phase V: v-projection for all heads, natural layout, spill ----
                        if "v" in phases:
                         with (
                            tc.tile_pool(name="wvp", bufs=1) as wvp,
                            tc.tile_pool(name="vtp", bufs=3) as vtp,
                        ):
                            wv_sb = [wvp.tile([128, HID], F32R, name=f"wv{t}") for t in range(HT)]
                            for t in range(HT):
                                nc.sync.dma_start(wv_sb[t][:], wv[128 * t:128 * (t + 1), :])
                            for tokt in range(KT):
                                vtile = vtp.tile([128, H * 65], F32R)
                                vview = vtile.rearrange("p (h c) -> p h c", c=65)
                                for vb in range(2):
                                    vps = psA.tile([128, 512], F32, tag="vps")
                                    for ht in range(HT):
                                        nc.tensor.matmul(
                                            vps[:],
                                            xt_sb[ht][:, 128 * tokt:128 * (tokt + 1)],
                                            wv_sb[ht][:, 512 * vb:512 * (vb + 1)],
                                            start=(ht == 0), stop=(ht == HT - 1),
                                        )
                                    nc.vector.tensor_copy(
                                        vview[:, 8 * vb:8 * (vb + 1), 0:64],
                                        vps.rearrange("p (h c) -> p h c", c=64),
                                    )
                                nc.vector.tensor_copy(vview[:, :, 64], on16_sb[:, 0:1].broadcast_to([128, 16]))
                                nc.sync.dma_start(
                                    vdram[128 * tokt:128 * (tokt + 1), :], vtile[:]
                                )

                        # ---- phase Q: q-projection, transposed layout ----
                        if "q" in phases:
                         with (
                            tc.tile_pool(name="xtqp", bufs=1) as xtqp,
                            tc.tile_pool(name="wqp", bufs=2) as wqp,
                        ):
                            xtq_sb = [xtqp.tile([128, SQ], F32R, name=f"xtq{t}") for t in range(HT)]
                            for t in range(HT):
                                nc.sync.dma_start(xtq_sb[t][:], xtq[128 * t:128 * (t + 1), :])
                            for pr in range(NP):
                                wq_p = wqp.tile([128, HID], F32R)
                                nc.sync.dma_start(
                                    wq_p.rearrange("p (t c) -> p t c", c=128),
                                    wq[HID * pr:HID * (pr + 1), :].rearrange("(t p) c -> p t c", p=128),
                                )
                                for qb in range(QB):
                                    qps = psA.tile([128, 512], F32, tag="vps")
                                    for ht in range(HT):
                                        nc.tensor.matmul(
                                            qps[:],
                                            wq_p[:, 128 * ht:128 * (ht + 1)],
                                            xtq_sb[ht][:, 512 * qb:512 * (qb + 1)],
                                            start=(ht == 0), stop=(ht == HT - 1),
                                        )
                                    nc.vector.tensor_copy(qt_sb[pr][:, 512 * qb:512 * (qb + 1)], qps[:])

                    # ---- pair loop: K.T proj + attention + normalize ----
                    if "pair" in phases:
                     with (
                        tc.tile_pool(name="wkp", bufs=2) as wkp,
                        tc.tile_pool(name="ktp", bufs=2) as ktp,
                        tc.tile_pool(name="vt2p", bufs=2) as vt2p,
                        tc.tile_pool(name="ptp", bufs=2) as ptp,
                        tc.tile_pool(name="nrm", bufs=2) as nrm,
                        tc.tile_pool(name="psB", bufs=2, space="PSUM") as psB,
                        tc.tile_pool(name="psL", bufs=2, space="PSUM") as psL,
                        tc.tile_pool(name="psV", bufs=2, space="PSUM") as psV,
                    ):
                        for pr in range(NP):
                            wk_p = wkp.tile([128, HID], F32R)
                            nc.sync.dma_start(
                                wk_p.rearrange("p (t c) -> p t c", c=128),
                                wk[HID * pr:HID * (pr + 1), :].rearrange("(t p) c -> p t c", p=128),
                            )
                            kt_sb = ktp.tile([128, S], F32R)
                            for tb in range(TB):
                                kps = psB.tile([128, 512], F32, tag="kps")
                                for ht in range(HT):
                                    nc.tensor.matmul(
                                        kps[:],
                                        wk_p[:, 128 * ht:128 * (ht + 1)],
                                        xt_sb[ht][:, 512 * tb:512 * (tb + 1)],
                                        start=(ht == 0), stop=(ht == HT - 1),
                                    )
                                nc.vector.tensor_copy(kt_sb[:, 512 * tb:512 * (tb + 1)], kps[:])

                            vpair = vt2p.tile([128, KT * 130], F32R)
                            nc.sync.dma_start(
                                vpair.rearrange("p (k c) -> p k c", c=130),
                                vdram.rearrange("(k p) c -> p k c", p=128)[:, :, 130 * pr:130 * (pr + 1)],
                            )

                            vtmp = nrm.tile([64, SQ], F32R, tag="vtmp")
                            for qb in range(QB):
                                vals = [psV.tile([65, 512], F32, tag="vals", name=f"vals{h}") for h in range(2)]
                                for k in range(KT):
                                    lg = psL.tile([128, 1024], F32, tag="lg")
                                    for h in range(2):
                                        nc.tensor.matmul(
                                            lg[:, 512 * h:512 * (h + 1)],
                                            kt_sb[64 * h:64 * (h + 1), 128 * k:128 * (k + 1)],
                                            qt_sb[pr][64 * h:64 * (h + 1), 512 * qb:512 * (qb + 1)],
                                            start=True, stop=True,
                                        )
                                    pt = ptp.tile([128, 1024], F32R)
                                    nc.scalar.activation(pt[:], lg[:], EXP, scale=0.125)
                                    for h in range(2):
                                        nc.tensor.matmul(
                                            vals[h][:],
                                            vpair[:, 130 * k + 65 * h:130 * k + 65 * (h + 1)],
                                            pt[:, 512 * h:512 * (h + 1)],
                                            start=(k == 0), stop=(k == KT - 1),
                                        )
                                for h in range(2):
                                    srow = nrm.tile([1, 512], F32R, tag="srow")
                                    nc.vector.tensor_copy(srow[:], vals[h][64:65, :])
                                    bc = psL.tile([64, 512], F32, tag="lg", name="bc")
                                    nc.tensor.matmul(bc[:], ones_sb[:], srow[:], start=True, stop=True)
                                    rec = nrm.tile([64, 512], F32, tag="rec")
                                    nc.vector.reciprocal(rec[:], bc[:])
                                    if h == 0:
                                        dst = vn_all[0:64, SQ * pr + 512 * qb:SQ * pr + 512 * (qb + 1)]
                                        nc.vector.tensor_mul(dst, vals[h][0:64, :], rec[:])
                                    else:
                                        nc.vector.tensor_mul(vtmp[:, 512 * qb:512 * (qb + 1)], vals[h][0:64, :], rec[:])
                            nc.sync.dma_start(vn_all[64:128, SQ * pr:SQ * (pr + 1)], vtmp[:])

                # ---- phase O: o-projection ----
                if "o" in phases:
                 with (
                    tc.tile_pool(name="wop", bufs=1) as wop,
                    tc.tile_pool(name="obp", bufs=2) as obp,
                    tc.tile_pool(name="psO", bufs=4, space="PSUM") as psO,
                ):
                    wo_sb = [wop.tile([128, HID], F32R, name=f"wo{t}") for t in range(HT)]
                    for t in range(HT):
                        nc.sync.dma_start(wo_sb[t][:], wo[128 * t:128 * (t + 1), :])
                    for tokb in range(SQ // 128):
                        o_sb = obp.tile([128, HID], F32)
                        for ob in range(2):
                            ops = psO.tile([128, 512], F32, tag="ops")
                            for t in range(HT):
                                nc.tensor.matmul(
                                    ops[:],
                                    vn_all[:, SQ * t + 128 * tokb:SQ * t + 128 * (tokb + 1)],
                                    wo_sb[t][:, 512 * ob:512 * (ob + 1)],
                                    start=(t == 0), stop=(t == HT - 1),
                                )
                            nc.vector.tensor_copy(o_sb[:, 512 * ob:512 * (ob + 1)], ops[:])
                        nc.sync.dma_start(o[128 * tokb:128 * (tokb + 1), :], o_sb[:])

        if n_iter > 1:
            with tc.For_i(0, n_iter, 1):
                body()
        else:
            body()

    nc.compile()
    return nc


def shard_inputs(x, w_qkv, w_o):
    x = np.asarray(x, dtype=np.float32)
    w_qkv = np.asarray(w_qkv, dtype=np.float32)
    w_o = np.asarray(w_o, dtype=np.float32)
    w3 = w_qkv.reshape(H, 3 * D, HID)
    wq_t = w3[:, 0:D, :].reshape(HID, HID).T      # [hid, of]
    wk_t = w3[:, D:2 * D, :].reshape(HID, HID).T
    wv_h = np.ascontiguousarray(w3[:, 2 * D:3 * D, :].reshape(HID, HID).T)
    # block [hid, of] -> [NP*hid, 128]: pair-major column blocks
    wq_h = np.ascontiguousarray(wq_t.reshape(HID, NP, 128).transpose(1, 0, 2).reshape(NP * HID, 128))
    wk_h = np.ascontiguousarray(wk_t.reshape(HID, NP, 128).transpose(1, 0, 2).reshape(NP * HID, 128))
    wo_h = np.ascontiguousarray(w_o.T)
    ones16 = np.ones((128, 16), np.float32)
    cone = np.ones((1, 64), np.float32)
    in_maps = []
    for core in range(N_CORES):
        b, half = core // 2, core % 2
        xt_b = np.ascontiguousarray(x[b].T)
        xtq_b = np.ascontiguousarray(x[b, SQ * half:SQ * (half + 1), :].T)
        in_maps.append({
            "xt": xt_b, "xtq": xtq_b,
            "wq": wq_h, "wk": wk_h, "wv": wv_h, "wo": wo_h,
            "ones16": ones16, "cone": cone,
        })
    return in_maps


_NC_CACHE = {}


def get_nc(n_iter: int = 1):
    if n_iter not in _NC_CACHE:
        _NC_CACHE[n_iter] = build_nc(n_iter)
    return _NC_CACHE[n_iter]


def kernel(x, w_qkv, w_o):
    nc = get_nc(1)
    in_maps = shard_inputs(x, w_qkv, w_o)
    res = run_bass_kernel_spmd(nc, in_maps, list(range(N_CORES)))
    out = np.empty((B, S, HID), np.float32)
    for core in range(N_CORES):
        b, half = core // 2, core % 2
        out[b, SQ * half:SQ * (half + 1), :] = res.results[core]["o"]
    return out



# revision 21
# speedup vs baseline: 1.1205x; 1.1205x over previous
"""Multi-head attention (B=4, S=2048, HID=1024, H=16, D=64) on 8 trn2 cores.

Sharding: batch x head-group (4 x 2). Core (2b+g) owns batch b and heads
8g..8g+7 over the FULL sequence: it computes Q/K/V projections for its 8
heads, attention, and a partial o-projection over its 512 value features.
The host sums the two partial o outputs per batch (the "all-reduce after
o_proj" done host-side) -- no duplicated projection work, no collectives.

Per-core dataflow:
  - x.T and w_qkv shards in bf16 (proj matmuls bf16, fp32 PSUM accumulate)
  - K.T/Q.T per head pair in [feature, token] f32r layout (feeds logits)
  - V' in natural [token, feature] f32r layout with a ones column per head
    (softmax denominator comes out of the AV matmul as row 64)
  - logits transposed per head: L.T[k, q] = K_h.T.T @ Q_h.T, head pairs at
    PE row-tiles 0/64; exp on the scalar engine over [128, 1024] tiles
  - AV accumulates vals'[65, 512] over 16 k-tiles; row 64 = denominator
  - denominator rows collected into a [32, 512] tile; ONE reciprocal per
    pair group (free-size bound, so batching rows is ~8x cheaper than
    reciprocal of broadcast tiles); PE broadcast + DVE mul to normalize
  - o_proj tail: vn (bf16) @ w_o.T shard (bf16), accumulated over 4
    feature chunks, streamed out as fp32
"""
import sys
sys.path.insert(0, "/opt/trn_rl_repo")
import numpy as np

import concourse.bass as bass
import concourse.mybir as mybir
import concourse.tile as tile
from concourse import bacc
from concourse.bass_utils import run_bass_kernel_spmd

F32 = mybir.dt.float32
F32R = mybir.dt.float32r
BF16 = mybir.dt.bfloat16
EXP = mybir.ActivationFunctionType.Exp

B, S, HID, H, D = 4, 2048, 1024, 16, 64
G = 2                  # head groups (cores per batch)
HG = H // G            # 8 heads per core
NPAIR = HG // 2        # 4 head pairs per core
HT = HID // 128        # 8 hid contraction tiles
TB = S // 512          # 4 proj token blocks
KT = S // 128          # 16 key-token tiles
QB = S // 512          # 4 query blocks of 512
N_CORES = 8


def build_nc(n_iter: int = 1):
    nc = bacc.Bacc(None, target_bir_lowering=False)

    xt = nc.dram_tensor("xt", [HID, S], BF16, kind="ExternalInput")
    wq = nc.dram_tensor("wq", [NPAIR * HID, 128], BF16, kind="ExternalInput")
    wk = nc.dram_tensor("wk", [NPAIR * HID, 128], BF16, kind="ExternalInput")
    wv = nc.dram_tensor("wv", [HID, HG * D], BF16, kind="ExternalInput")
    wo = nc.dram_tensor("wo", [HG * D, HID], BF16, kind="ExternalInput")
    cone = nc.dram_tensor("cone", [1, 64], F32R, kind="ExternalInput")
    cone8 = nc.dram_tensor("cone8", [128, 8], BF16, kind="ExternalInput")
    o = nc.dram_tensor("o", [S, HID], F32, kind="ExternalOutput")

    with tile.TileContext(nc) as tc:
        def body():
            with (
                tc.tile_pool(name="const", bufs=1) as constp,
                tc.tile_pool(name="xtp", bufs=1) as xtp,
                tc.tile_pool(name="vtp", bufs=1) as vtp,
                tc.tile_pool(name="vnp", bufs=1) as vnp,
                tc.tile_pool(name="wop", bufs=1) as wop,
            ):
                ones_sb = constp.tile([1, 64], F32R)
                ones8_sb = constp.tile([128, 8], BF16)
                nc.sync.dma_start(ones_sb[:], cone[:])
                nc.sync.dma_start(ones8_sb[:], cone8[:])

                xt_sb = [xtp.tile([128, S], BF16, name=f"xt{t}") for t in range(HT)]
                for t in range(HT):
                    nc.sync.dma_start(xt_sb[t][:], xt[128 * t:128 * (t + 1), :])
                wo_sb = [wop.tile([128, HID], BF16, name=f"wo{c}") for c in range(NPAIR)]
                for c in range(NPAIR):
                    nc.sync.dma_start(wo_sb[c][:], wo[128 * c:128 * (c + 1), :])

                # V' [token, (kt, head, 65)] bf16, resident in SBUF
                vt = vtp.tile([128, KT * HG * 65], BF16)
                vt4 = vt.rearrange("p (t h c) -> p t h c", h=HG, c=65)
                # normalized values [feat(128=2 heads), pair-chunk, token] bf16
                vn_all = vnp.tile([128, NPAIR * S], BF16)

                with (
                    tc.tile_pool(name="wvp", bufs=1) as wvp,
                    tc.tile_pool(name="ktp", bufs=2) as ktp,
                    tc.tile_pool(name="qtp", bufs=2) as qtp,
                    tc.tile_pool(name="wkp", bufs=2) as wkp,
                    tc.tile_pool(name="wqp", bufs=2) as wqp,
                    tc.tile_pool(name="ptp", bufs=3) as ptp,
                    tc.tile_pool(name="vap", bufs=10) as vap,
                    tc.tile_pool(name="nrm", bufs=2) as nrm,
                    tc.tile_pool(name="dramp", bufs=2, space="DRAM") as dramp,
                    tc.tile_pool(name="psP", bufs=2, space="PSUM") as psP,
                    tc.tile_pool(name="psL", bufs=2, space="PSUM") as psL,
                    tc.tile_pool(name="psV", bufs=2, space="PSUM") as psV,
                ):
                    # ---- V projection for all 8 heads ----
                    wv_sb = [wvp.tile([128, HG * D], BF16, name=f"wv{t}") for t in range(HT)]
                    for t in range(HT):
                        nc.sync.dma_start(wv_sb[t][:], wv[128 * t:128 * (t + 1), :])
                    for tokt in range(KT):
                        vps = psP.tile([128, 512], F32, tag="pp")
                        for ht in range(HT):
                            nc.tensor.matmul(
                                vps[:],
                                xt_sb[ht][:, 128 * tokt:128 * (tokt + 1)],
                                wv_sb[ht][:],
                                start=(ht == 0), stop=(ht == HT - 1),
                            )
                        nc.vector.tensor_copy(
                            vt4[:, tokt, :, 0:64],
                            vps.rearrange("p (h c) -> p h c", c=64),
                        )
                        nc.vector.tensor_copy(vt4[:, tokt, :, 64], ones8_sb[:])

                    # ---- pair pipeline: K/Q proj + attention + normalize ----
                    va_tiles = {}

                    def proj_pair(j, w_dram, pool, dst):
                        w_p = pool.tile([128, HID], BF16, tag="wp")
                        nc.sync.dma_start(
                            w_p.rearrange("p (t c) -> p t c", c=128),
                            w_dram[HID * j:HID * (j + 1), :].rearrange("(t p) c -> p t c", p=128),
                        )
                        for tb in range(TB):
                            pps = psP.tile([128, 512], F32, tag="pp")
                            for ht in range(HT):
                                nc.tensor.matmul(
                                    pps[:],
                                    w_p[:, 128 * ht:128 * (ht + 1)],
                                    xt_sb[ht][:, 512 * tb:512 * (tb + 1)],
                                    start=(ht == 0), stop=(ht == HT - 1),
                                )
                            nc.vector.tensor_copy(dst[:, 512 * tb:512 * (tb + 1)], pps[:])

                    def attn_block(j, h2, qb, kt_sb, qt_sb, den_flat):
                        h = 2 * j + h2
                        vals = psV.tile([65, 512], F32, tag="vv")
                        for ktp2 in range(KT // 2):
                            lg = psL.tile([128, 1024], F32, tag="lg")
                            for u in range(2):
                                kt = 2 * ktp2 + u
                                nc.tensor.matmul(
                                    lg[:, 512 * u:512 * (u + 1)],
                                    kt_sb[64 * h2:64 * (h2 + 1), 128 * kt:128 * (kt + 1)],
                                    qt_sb[64 * h2:64 * (h2 + 1), 512 * qb:512 * (qb + 1)],
                                    start=True, stop=True,
                                )
                            pt = ptp.tile([128, 1024], BF16, tag="pt")
                            nc.scalar.activation(pt[:], lg[:], EXP, scale=0.125)
                            for u in range(2):
                                kt = 2 * ktp2 + u
                                nc.tensor.matmul(
                                    vals[:],
                                    vt4[:, kt, h, :],
                                    pt[:, 512 * u:512 * (u + 1)],
                                    start=(ktp2 == 0 and u == 0),
                                    stop=(ktp2 == KT // 2 - 1 and u == 1),
                                )
                        r = 4 * h2 + qb
                        nc.vector.tensor_copy(den_flat[0:1, 512 * r:512 * (r + 1)], vals[64:65, :])
                        va = vap.tile([64, 512], F32R, tag="va")
                        nc.vector.tensor_copy(va[:], vals[0:64, :])
                        va_tiles[8 * j + r] = va

                    def normalize(j, h2, qb, rec_flat):
                        r = 4 * h2 + qb
                        bc = psP.tile([64, 512], F32, tag="pp", name="bc")
                        nc.tensor.matmul(
                            bc[:], ones_sb[:], rec_flat[0:1, 512 * r:512 * (r + 1)],
                            start=True, stop=True,
                        )
                        nc.vector.tensor_mul(
                            vn_all[64 * h2:64 * (h2 + 1), S * j + 512 * qb:S * j + 512 * (qb + 1)],
                            va_tiles[8 * j + r][:],
                            bc[:],
                        )

                    for j in range(NPAIR):
                        kt_sb = ktp.tile([128, S], F32R, tag="kt")
                        qt_sb = qtp.tile([128, S], F32R, tag="qt")
                        den_flat = nrm.tile([1, 8 * 512], BF16, tag="dflat")
                        proj_pair(j, wk, wkp, kt_sb)
                        proj_pair(j, wq, wqp, qt_sb)
                        for h2 in range(2):
                            for qb in range(QB):
                                attn_block(j, h2, qb, kt_sb, qt_sb, den_flat)
                        # reciprocal of this pair's 8 denominator rows:
                        # bounce the flat row through DRAM to partition-major,
                        # one cheap free-size-512 reciprocal, bounce back flat
                        # for the PE broadcast (engine ops can't address
                        # partitions 1..31)
                        den_dram = dramp.tile([8, 512], BF16, tag="ddram")
                        nc.sync.dma_start(
                            den_dram.rearrange("r c -> (r c)")[None, :], den_flat[0:1, :]
                        )
                        den_sq = nrm.tile([8, 512], BF16, tag="dsq")
                        nc.sync.dma_start(den_sq[:], den_dram[:])
                        rec_sq = nrm.tile([8, 512], F32R, tag="rsq")
                        with nc.allow_low_precision(reason="f32r is fp32 bits; PE-input tag only"):
                            nc.vector.reciprocal(rec_sq[:], den_sq[:])
                        rec_dram = dramp.tile([8, 512], F32R, tag="rdram")
                        nc.sync.dma_start(rec_dram[:], rec_sq[:])
                        rec_flat = nrm.tile([1, 8 * 512], F32R, tag="rflat")
                        nc.sync.dma_start(
                            rec_flat[0:1, :], rec_dram.rearrange("r c -> (r c)")[None, :]
                        )
                        for h2 in range(2):
                            for qb in range(QB):
                                normalize(j, h2, qb, rec_flat)

                # ---- o projection tail ----
                with (
                    tc.tile_pool(name="obp", bufs=2) as obp,
                    tc.tile_pool(name="psO", bufs=2, space="PSUM") as psO,
                ):
                    vn3 = vn_all.rearrange("p (c s) -> p c s", c=NPAIR)
                    for tokb in range(S // 128):
                        o_sb = obp.tile([128, HID], F32)
                        for ob in range(2):
                            ops = psO.tile([128, 512], F32, tag="oo")
                            for c in range(NPAIR):
                                nc.tensor.matmul(
                                    ops[:],
                                    vn3[:, c, 128 * tokb:128 * (tokb + 1)],
                                    wo_sb[c][:, 512 * ob:512 * (ob + 1)],
                                    start=(c == 0), stop=(c == NPAIR - 1),
                                )
                            nc.vector.tensor_copy(o_sb[:, 512 * ob:512 * (ob + 1)], ops[:])
                        nc.sync.dma_start(o[128 * tokb:128 * (tokb + 1), :], o_sb[:])

        if n_iter > 1:
            with tc.For_i(0, n_iter, 1):
                body()
        else:
            body()

    nc.compile()
    return nc


def shard_inputs(x, w_qkv, w_o):
    x = np.asarray(x, dtype=np.float32)
    w_qkv = np.asarray(w_qkv, dtype=np.float32)
    w_o = np.asarray(w_o, dtype=np.float32)
    import ml_dtypes
    bf = ml_dtypes.bfloat16

    # w_qkv row (h*192 + c): c<64 q, 64<=c<128 k, 128<=c<192 v
    w3 = w_qkv.reshape(H, 3 * D, HID)
    wq_h = w3[:, 0:D, :]        # [H, D, HID]
    wk_h = w3[:, D:2 * D, :]
    wv_h = w3[:, 2 * D:3 * D, :]
    wo_t = w_o.T                # [HID(vals feat, h-major), HID(out)]

    cone = np.ones((1, 64), np.float32)
    cone8 = np.ones((128, 8), np.float32).astype(bf)
    in_maps = []
    for core in range(N_CORES):
        b, g = core // G, core % G
        hsel = slice(HG * g, HG * (g + 1))
        # [NPAIR*HID, 128]: pair-major blocks of W.T with 2 heads side by side
        wq_g = wq_h[hsel].reshape(NPAIR, 2 * D, HID).transpose(0, 2, 1).reshape(NPAIR * HID, 128)
        wk_g = wk_h[hsel].reshape(NPAIR, 2 * D, HID).transpose(0, 2, 1).reshape(NPAIR * HID, 128)
        wv_g = wv_h[hsel].reshape(HG * D, HID).T        # [HID, 512]
        wo_g = wo_t[HG * D * g:HG * D * (g + 1), :]     # [512, HID]
        in_maps.append({
            "xt": np.ascontiguousarray(x[b].T).astype(bf),
            "wq": np.ascontiguousarray(wq_g).astype(bf),
            "wk": np.ascontiguousarray(wk_g).astype(bf),
            "wv": np.ascontiguousarray(wv_g).astype(bf),
            "wo": np.ascontiguousarray(wo_g).astype(bf),
            "cone": cone, "cone8": cone8,
        })
    return in_maps


_NC_CACHE = {}


def get_nc(n_iter: int = 1):
    if n_iter not in _NC_CACHE:
        _NC_CACHE[n_iter] = build_nc(n_iter)
    return _NC_CACHE[n_iter]


def kernel(x, w_qkv, w_o):
    nc = get_nc(1)
    in_maps = shard_inputs(x, w_qkv, w_o)
    res = run_bass_kernel_spmd(nc, in_maps, list(range(N_CORES)))
    out = np.empty((B, S, HID), np.float32)
    for b in range(B):
        out[b] = res.results[G * b]["o"]
        for g in range(1, G):
            out[b] += res.results[G * b + g]["o"]
    return out


# revision 26
# speedup vs baseline: 1.4906x; 1.3303x over previous
"""Multi-head attention (B=4, S=2048, HID=1024, H=16, D=64) on 8 trn2 cores.

Sharding: batch x head-group (4 x 2). Core (2b+g) owns batch b and heads
8g..8g+7 over the FULL sequence: Q/K/V projections for its 8 heads,
attention, and a partial o-projection over its 512 value features. The host
sums the two partial o outputs per batch (the "all-reduce after o_proj"
done host-side) -- no duplicated projection work, no collectives.

Per-core dataflow (all matmuls full 128-partition moving operands, bf16,
fp32 PSUM accumulate -- avoids the half-bandwidth 64-partition moving path
and PE tiling-mode-switch drains):
  - K.T per pair packed [128=2x64 feat, token] bf16
  - Q.T per head zero-padded to [128, token] bf16 (other head's rows = 0),
    so logits contract over 128 partitions with the packed K stationary
  - V' in [token, (kt, head, 65)] bf16 with a ones column per head
    (softmax denominator falls out of the AV matmul as row 64)
  - logits L.T[k, q] in PSUM [128, 1024] (2 k-tiles); exp on ScalarE
  - AV accumulates vals'[65, 512] over 16 k-tiles; row 64 = denominator
  - denominator rows DMA'd from PSUM into a partition-major [8, 512] tile;
    ONE reciprocal per pair (free-size bound: 8x cheaper than reciprocal of
    broadcast tiles); bounced via DRAM back to a flat row, PE-broadcast,
    DVE multiply into vn
  - o_proj tail: vn (bf16) @ w_o.T shard (bf16) over 4 feature chunks
"""
import sys
sys.path.insert(0, "/opt/trn_rl_repo")
import numpy as np

import concourse.bass as bass
import concourse.mybir as mybir
import concourse.tile as tile
from concourse import bacc
from concourse.bass_utils import run_bass_kernel_spmd

F32 = mybir.dt.float32
F32R = mybir.dt.float32r
BF16 = mybir.dt.bfloat16
EXP = mybir.ActivationFunctionType.Exp

B, S, HID, H, D = 4, 2048, 1024, 16, 64
G = 2                  # head groups (cores per batch)
HG = H // G            # 8 heads per core
NPAIR = HG // 2        # 4 head pairs per core
HT = HID // 128        # 8 hid contraction tiles
TB = S // 512          # 4 proj token blocks
KT = S // 128          # 16 key-token tiles
QB = S // 512          # 4 query blocks of 512
N_CORES = 8


def build_nc(n_iter: int = 1):
    nc = bacc.Bacc(None, target_bir_lowering=False)

    xt = nc.dram_tensor("xt", [HID, S], BF16, kind="ExternalInput")
    wq = nc.dram_tensor("wq", [NPAIR * HID, 128], BF16, kind="ExternalInput")
    wk = nc.dram_tensor("wk", [NPAIR * HID, 128], BF16, kind="ExternalInput")
    wv = nc.dram_tensor("wv", [HID, HG * D], BF16, kind="ExternalInput")
    wo = nc.dram_tensor("wo", [HG * D, HID], BF16, kind="ExternalInput")
    cone = nc.dram_tensor("cone", [1, 64], BF16, kind="ExternalInput")
    cone8 = nc.dram_tensor("cone8", [128, 8], BF16, kind="ExternalInput")
    o = nc.dram_tensor("o", [S, HID], F32, kind="ExternalOutput")

    with tile.TileContext(nc) as tc:
        def body():
            with (
                tc.tile_pool(name="const", bufs=1) as constp,
                tc.tile_pool(name="xtp", bufs=1) as xtp,
                tc.tile_pool(name="vtp", bufs=1) as vtp,
                tc.tile_pool(name="vnp", bufs=1) as vnp,
                tc.tile_pool(name="wop", bufs=1) as wop,
                tc.tile_pool(name="ktqp", bufs=1) as ktqp,
            ):
                ones_sb = constp.tile([1, 64], BF16)
                ones8_sb = constp.tile([128, 8], BF16)
                nc.sync.dma_start(ones_sb[:], cone[:])
                nc.sync.dma_start(ones8_sb[:], cone8[:])

                xt_sb = [xtp.tile([128, S], BF16, name=f"xt{t}") for t in range(HT)]
                for t in range(HT):
                    nc.sync.dma_start(xt_sb[t][:], xt[128 * t:128 * (t + 1), :])
                wo_sb = [wop.tile([128, HID], BF16, name=f"wo{c}") for c in range(NPAIR)]
                for c in range(NPAIR):
                    nc.sync.dma_start(wo_sb[c][:], wo[128 * c:128 * (c + 1), :])

                # V' [token, (kt, head, 65)] bf16, resident in SBUF
                vt = vtp.tile([128, KT * HG * 65], BF16)
                vt4 = vt.rearrange("p (t h c) -> p t h c", h=HG, c=65)
                # normalized values [feat(128=2 heads), pair-chunk, token]
                vn_all = vnp.tile([128, NPAIR * S], BF16)

                # persistent K/Q tiles, double-buffered across pairs.
                # qt_h zero-halves are memset once and never overwritten.
                kt_t = [ktqp.tile([128, S], BF16, name=f"kt{i}") for i in range(2)]
                qt_t = [[ktqp.tile([128, S], BF16, name=f"qt{i}{h2}") for h2 in range(2)]
                        for i in range(2)]
                for i in range(2):
                    nc.any.memset(qt_t[i][0][64:128, :], 0.0)
                    nc.any.memset(qt_t[i][1][0:64, :], 0.0)

                with (
                    tc.tile_pool(name="wvp", bufs=1) as wvp,
                    tc.tile_pool(name="wkp", bufs=2) as wkp,
                    tc.tile_pool(name="wqp", bufs=2) as wqp,
                    tc.tile_pool(name="ptp", bufs=3) as ptp,
                    tc.tile_pool(name="vap", bufs=12) as vap,
                    tc.tile_pool(name="nrm", bufs=2) as nrm,
                    tc.tile_pool(name="dramp", bufs=2, space="DRAM") as dramp,
                    tc.tile_pool(name="psP", bufs=2, space="PSUM") as psP,
                    tc.tile_pool(name="psL", bufs=2, space="PSUM") as psL,
                    tc.tile_pool(name="psV", bufs=2, space="PSUM") as psV,
                ):
                    # ---- V projection for all 8 heads ----
                    wv_sb = [wvp.tile([128, HG * D], BF16, name=f"wv{t}") for t in range(HT)]
                    for t in range(HT):
                        nc.sync.dma_start(wv_sb[t][:], wv[128 * t:128 * (t + 1), :])
                    for tokt in range(KT):
                        vps = psP.tile([128, 512], F32, tag="pp")
                        for ht in range(HT):
                            nc.tensor.matmul(
                                vps[:],
                                xt_sb[ht][:, 128 * tokt:128 * (tokt + 1)],
                                wv_sb[ht][:],
                                start=(ht == 0), stop=(ht == HT - 1),
                            )
                        nc.vector.tensor_copy(
                            vt4[:, tokt, :, 0:64],
                            vps.rearrange("p (h c) -> p h c", c=64),
                        )
                        nc.vector.tensor_copy(vt4[:, tokt, :, 64], ones8_sb[:])

                    # ---- pair pipeline: K/Q proj + attention + normalize ----
                    va_tiles = {}

                    def proj_pair(j, w_dram, pool, evac):
                        w_p = pool.tile([128, HID], BF16, tag="wp")
                        nc.sync.dma_start(
                            w_p.rearrange("p (t c) -> p t c", c=128),
                            w_dram[HID * j:HID * (j + 1), :].rearrange("(t p) c -> p t c", p=128),
                        )
                        for tb in range(TB):
                            pps = psP.tile([128, 512], F32, tag="pp")
                            for ht in range(HT):
                                nc.tensor.matmul(
                                    pps[:],
                                    w_p[:, 128 * ht:128 * (ht + 1)],
                                    xt_sb[ht][:, 512 * tb:512 * (tb + 1)],
                                    start=(ht == 0), stop=(ht == HT - 1),
                                )
                            evac(tb, pps)

                    def attn_block(j, h2, qb, kt_sb, qt_sb, den_flat):
                        h = 2 * j + h2
                        vals = psV.tile([65, 512], F32, tag="vv")
                        for ktp2 in range(KT // 2):
                            lg = psL.tile([128, 1024], F32, tag="lg")
                            for u in range(2):
                                kt = 2 * ktp2 + u
                                nc.tensor.matmul(
                                    lg[:, 512 * u:512 * (u + 1)],
                                    kt_sb[:, 128 * kt:128 * (kt + 1)],
                                    qt_sb[:, 512 * qb:512 * (qb + 1)],
                                    start=True, stop=True,
                                )
                            pt = ptp.tile([128, 1024], BF16, tag="pt")
                            nc.scalar.activation(pt[:], lg[:], EXP, scale=0.125)
                            for u in range(2):
                                kt = 2 * ktp2 + u
                                nc.tensor.matmul(
                                    vals[:],
                                    vt4[:, kt, h, :],
                                    pt[:, 512 * u:512 * (u + 1)],
                                    start=(ktp2 == 0 and u == 0),
                                    stop=(ktp2 == KT // 2 - 1 and u == 1),
                                )
                        r = 4 * h2 + qb
                        nc.vector.tensor_copy(den_flat[0:1, 512 * r:512 * (r + 1)], vals[64:65, :])
                        va = vap.tile([64, 512], F32R, tag="va")
                        nc.vector.tensor_copy(va[:], vals[0:64, :])
                        va_tiles[8 * j + r] = va

                    for j in range(NPAIR):
                        kt_sb = kt_t[j % 2]
                        den_flat = nrm.tile([1, 8 * 512], BF16, tag="dflat")

                        def k_evac(tb, pps):
                            nc.vector.tensor_copy(kt_sb[:, 512 * tb:512 * (tb + 1)], pps[:])

                        def q_evac(tb, pps):
                            nc.vector.tensor_copy(
                                qt_t[j % 2][0][0:64, 512 * tb:512 * (tb + 1)], pps[0:64, :])
                            nc.vector.tensor_copy(
                                qt_t[j % 2][1][64:128, 512 * tb:512 * (tb + 1)], pps[64:128, :])

                        proj_pair(j, wk, wkp, k_evac)
                        proj_pair(j, wq, wqp, q_evac)
                        for h2 in range(2):
                            for qb in range(QB):
                                attn_block(j, h2, qb, kt_sb, qt_t[j % 2][h2], den_flat)

                        # one cheap reciprocal for the pair's 8 denominator rows,
                        # bounced through DRAM to partition-major and back to a
                        # flat partition-0 row (engine ops cannot address
                        # partitions 1..31)
                        den_dram = dramp.tile([8, 512], BF16, tag="ddram")
                        nc.sync.dma_start(
                            den_dram.rearrange("r c -> (r c)")[None, :], den_flat[0:1, :]
                        )
                        den_sq = nrm.tile([8, 512], BF16, tag="dsq")
                        nc.sync.dma_start(den_sq[:], den_dram[:])
                        rec_sq = nrm.tile([8, 512], BF16, tag="rsq")
                        with nc.allow_low_precision(reason="denominator reciprocal in bf16"):
                            nc.vector.reciprocal(rec_sq[:], den_sq[:])
                        rec_dram = dramp.tile([8, 512], BF16, tag="rdram")
                        nc.sync.dma_start(rec_dram[:], rec_sq[:])
                        rec_flat = nrm.tile([1, 8 * 512], BF16, tag="rflat")
                        nc.sync.dma_start(
                            rec_flat[0:1, :], rec_dram.rearrange("r c -> (r c)")[None, :]
                        )
                        for h2 in range(2):
                            for qb in range(QB):
                                r = 4 * h2 + qb
                                bc = psL.tile([64, 512], F32, tag="lg", name="bc")
                                nc.tensor.matmul(
                                    bc[:], ones_sb[:], rec_flat[0:1, 512 * r:512 * (r + 1)],
                                    start=True, stop=True,
                                )
                                nc.vector.tensor_mul(
                                    vn_all[64 * h2:64 * (h2 + 1),
                                           S * j + 512 * qb:S * j + 512 * (qb + 1)],
                                    va_tiles[8 * j + r][:],
                                    bc[:],
                                )

                # ---- o projection tail ----
                with (
                    tc.tile_pool(name="obp", bufs=2) as obp,
                    tc.tile_pool(name="psO", bufs=2, space="PSUM") as psO,
                ):
                    vn3 = vn_all.rearrange("p (c s) -> p c s", c=NPAIR)
                    for tokb in range(S // 128):
                        o_sb = obp.tile([128, HID], F32)
                        for ob in range(2):
                            ops = psO.tile([128, 512], F32, tag="oo")
                            for c in range(NPAIR):
                                nc.tensor.matmul(
                                    ops[:],
                                    vn3[:, c, 128 * tokb:128 * (tokb + 1)],
                                    wo_sb[c][:, 512 * ob:512 * (ob + 1)],
                                    start=(c == 0), stop=(c == NPAIR - 1),
                                )
                            nc.vector.tensor_copy(o_sb[:, 512 * ob:512 * (ob + 1)], ops[:])
                        nc.sync.dma_start(o[128 * tokb:128 * (tokb + 1), :], o_sb[:])

        if n_iter > 1:
            with tc.For_i(0, n_iter, 1):
                body()
        else:
            body()

    nc.compile()
    return nc


def shard_inputs(x, w_qkv, w_o):
    x = np.asarray(x, dtype=np.float32)
    w_qkv = np.asarray(w_qkv, dtype=np.float32)
    w_o = np.asarray(w_o, dtype=np.float32)
    import ml_dtypes
    bf = ml_dtypes.bfloat16

    # w_qkv row (h*192 + c): c<64 q, 64<=c<128 k, 128<=c<192 v
    w3 = w_qkv.reshape(H, 3 * D, HID)
    wq_h = w3[:, 0:D, :]        # [H, D, HID]
    wk_h = w3[:, D:2 * D, :]
    wv_h = w3[:, 2 * D:3 * D, :]
    wo_t = w_o.T                # [HID(vals feat, h-major), HID(out)]

    cone = np.ones((1, 64), np.float32).astype(bf)
    cone8 = np.ones((128, 8), np.float32).astype(bf)
    in_maps = []
    for core in range(N_CORES):
        b, g = core // G, core % G
        hsel = slice(HG * g, HG * (g + 1))
        wq_g = wq_h[hsel].reshape(NPAIR, 2 * D, HID).transpose(0, 2, 1).reshape(NPAIR * HID, 128)
        wk_g = wk_h[hsel].reshape(NPAIR, 2 * D, HID).transpose(0, 2, 1).reshape(NPAIR * HID, 128)
        wv_g = wv_h[hsel].reshape(HG * D, HID).T        # [HID, 512]
        wo_g = wo_t[HG * D * g:HG * D * (g + 1), :]     # [512, HID]
        in_maps.append({
            "xt": np.ascontiguousarray(x[b].T).astype(bf),
            "wq": np.ascontiguousarray(wq_g).astype(bf),
            "wk": np.ascontiguousarray(wk_g).astype(bf),
            "wv": np.ascontiguousarray(wv_g).astype(bf),
            "wo": np.ascontiguousarray(wo_g).astype(bf),
            "cone": cone, "cone8": cone8,
        })
    return in_maps


_NC_CACHE = {}


def get_nc(n_iter: int = 1):
    if n_iter not in _NC_CACHE:
        _NC_CACHE[n_iter] = build_nc(n_iter)
    return _NC_CACHE[n_iter]


def kernel(x, w_qkv, w_o):
    nc = get_nc(1)
    in_maps = shard_inputs(x, w_qkv, w_o)
    res = run_bass_kernel_spmd(nc, in_maps, list(range(N_CORES)))
    out = np.empty((B, S, HID), np.float32)
    for b in range(B):
        out[b] = res.results[G * b]["o"]
        for g in range(1, G):
            out[b] += res.results[G * b + g]["o"]
    return out


# revision 31
# speedup vs baseline: 1.7431x; 1.1694x over previous
"""Multi-head attention (B=4, S=2048, HID=1024, H=16, D=64) on 8 trn2 cores.

Sharding: batch x head-group (4 x 2). Core (2b+g) owns batch b and heads
8g..8g+7 over the FULL sequence: Q/K/V projections for its 8 heads,
attention, and a partial o-projection over its 512 value features. The host
sums the two partial o outputs per batch (the "all-reduce after o_proj"
done host-side) -- no duplicated projection work, no collectives.

Per-core dataflow (all matmuls full 128-partition moving operands, bf16,
fp32 PSUM accumulate -- avoids the half-bandwidth 64-partition moving path
and PE tiling-mode-switch drains):
  - K.T per pair packed [128=2x64 feat, token] bf16
  - Q.T per head zero-padded to [128, token] bf16 (other head's rows = 0),
    so logits contract over 128 partitions with the packed K stationary
  - V' in [token, (kt, head, 65)] bf16 with a ones column per head
    (softmax denominator falls out of the AV matmul as row 64)
  - logits L.T[k, q] in PSUM [128, 1024] (2 k-tiles); exp on ScalarE
  - AV accumulates vals'[65, 512] over 16 k-tiles; row 64 = denominator
  - denominator rows DMA'd from PSUM into a partition-major [8, 512] tile;
    ONE reciprocal per pair (free-size bound: 8x cheaper than reciprocal of
    broadcast tiles); bounced via DRAM back to a flat row, PE-broadcast,
    DVE multiply into vn
  - o_proj tail: vn (bf16) @ w_o.T shard (bf16) over 4 feature chunks
"""
import sys
sys.path.insert(0, "/opt/trn_rl_repo")
import numpy as np

import concourse.bass as bass
import concourse.mybir as mybir
import concourse.tile as tile
from concourse import bacc
from concourse.bass_utils import run_bass_kernel_spmd

F32 = mybir.dt.float32
F32R = mybir.dt.float32r
BF16 = mybir.dt.bfloat16
EXP = mybir.ActivationFunctionType.Exp

B, S, HID, H, D = 4, 2048, 1024, 16, 64
G = 2                  # head groups (cores per batch)
HG = H // G            # 8 heads per core
NPAIR = HG // 2        # 4 head pairs per core
HT = HID // 128        # 8 hid contraction tiles
TB = S // 512          # 4 proj token blocks
KT = S // 128          # 16 key-token tiles
QB = S // 512          # 4 query blocks of 512
N_CORES = 8


def build_nc(n_iter: int = 1):
    nc = bacc.Bacc(None, target_bir_lowering=False)

    xt = nc.dram_tensor("xt", [HID, S], BF16, kind="ExternalInput")
    wq = nc.dram_tensor("wq", [NPAIR * HID, 128], BF16, kind="ExternalInput")
    wk = nc.dram_tensor("wk", [NPAIR * HID, 128], BF16, kind="ExternalInput")
    wv = nc.dram_tensor("wv", [HID, HG * D], BF16, kind="ExternalInput")
    wo = nc.dram_tensor("wo", [HG * D, HID], BF16, kind="ExternalInput")
    cone8 = nc.dram_tensor("cone8", [128, 8], BF16, kind="ExternalInput")
    o = nc.dram_tensor("o", [S, HID], F32, kind="ExternalOutput")

    with tile.TileContext(nc) as tc:
        def body():
            with (
                tc.tile_pool(name="const", bufs=1) as constp,
                tc.tile_pool(name="xtp", bufs=1) as xtp,
                tc.tile_pool(name="vtp", bufs=1) as vtp,
                tc.tile_pool(name="vnp", bufs=1) as vnp,
                tc.tile_pool(name="wop", bufs=1) as wop,
                tc.tile_pool(name="ktqp", bufs=1) as ktqp,
            ):
                ones8_sb = constp.tile([128, 8], BF16)
                nc.sync.dma_start(ones8_sb[:], cone8[:])

                xt_sb = [xtp.tile([128, S], BF16, name=f"xt{t}") for t in range(HT)]
                for t in range(HT):
                    nc.sync.dma_start(xt_sb[t][:], xt[128 * t:128 * (t + 1), :])
                wo_sb = [wop.tile([128, HID], BF16, name=f"wo{c}") for c in range(NPAIR)]
                for c in range(NPAIR):
                    nc.sync.dma_start(wo_sb[c][:], wo[128 * c:128 * (c + 1), :])

                # V' [token, (kt, head, 65)] bf16, resident in SBUF
                vt = vtp.tile([128, KT * HG * 65], BF16)
                vt4 = vt.rearrange("p (t h c) -> p t h c", h=HG, c=65)
                # normalized values [feat(128=2 heads), pair-chunk, token]
                vn_all = vnp.tile([128, NPAIR * S], BF16)

                # persistent K/Q tiles, double-buffered across pairs.
                # qt_h zero-halves are memset once and never overwritten.
                kt_t = [ktqp.tile([128, S], BF16, name=f"kt{i}") for i in range(2)]
                qt_t = [[ktqp.tile([128, S], BF16, name=f"qt{i}{h2}") for h2 in range(2)]
                        for i in range(2)]
                for i in range(2):
                    nc.any.memset(qt_t[i][0][64:128, :], 0.0)
                    nc.any.memset(qt_t[i][1][0:64, :], 0.0)

                with (
                    tc.tile_pool(name="wvp", bufs=1) as wvp,
                    tc.tile_pool(name="wkp", bufs=2) as wkp,
                    tc.tile_pool(name="wqp", bufs=2) as wqp,
                    tc.tile_pool(name="ptp", bufs=3) as ptp,
                    tc.tile_pool(name="vap", bufs=18) as vap,
                    tc.tile_pool(name="nrm", bufs=2) as nrm,
                    tc.tile_pool(name="dramp", bufs=2, space="DRAM") as dramp,
                    tc.tile_pool(name="psP", bufs=2, space="PSUM") as psP,
                    tc.tile_pool(name="psL", bufs=2, space="PSUM") as psL,
                    tc.tile_pool(name="psV", bufs=2, space="PSUM") as psV,
                ):
                    # ---- V projection for all 8 heads ----
                    wv_sb = [wvp.tile([128, HG * D], BF16, name=f"wv{t}") for t in range(HT)]
                    for t in range(HT):
                        nc.sync.dma_start(wv_sb[t][:], wv[128 * t:128 * (t + 1), :])
                    for tokt in range(KT):
                        vps = psP.tile([128, 512], F32, tag="pp")
                        for ht in range(HT):
                            nc.tensor.matmul(
                                vps[:],
                                xt_sb[ht][:, 128 * tokt:128 * (tokt + 1)],
                                wv_sb[ht][:],
                                start=(ht == 0), stop=(ht == HT - 1),
                            )
                        nc.vector.tensor_copy(
                            vt4[:, tokt, :, 0:64],
                            vps.rearrange("p (h c) -> p h c", c=64),
                        )
                        nc.vector.tensor_copy(vt4[:, tokt, :, 64], ones8_sb[:])

                    # ---- pair pipeline: K/Q proj + attention + normalize ----
                    va_tiles = {}

                    def proj_pair(j, w_dram, pool, evac):
                        w_p = pool.tile([128, HID], BF16, tag="wp")
                        nc.sync.dma_start(
                            w_p.rearrange("p (t c) -> p t c", c=128),
                            w_dram[HID * j:HID * (j + 1), :].rearrange("(t p) c -> p t c", p=128),
                        )
                        for tb in range(TB):
                            pps = psP.tile([128, 512], F32, tag="pp")
                            for ht in range(HT):
                                nc.tensor.matmul(
                                    pps[:],
                                    w_p[:, 128 * ht:128 * (ht + 1)],
                                    xt_sb[ht][:, 512 * tb:512 * (tb + 1)],
                                    start=(ht == 0), stop=(ht == HT - 1),
                                )
                            evac(tb, pps)

                    def attn_block(j, h2, qb, kt_sb, qt_sb, den_flat):
                        h = 2 * j + h2
                        vals = psV.tile([65, 512], F32, tag="vv")
                        for ktp2 in range(KT // 2):
                            lg = psL.tile([128, 1024], F32, tag="lg")
                            for u in range(2):
                                kt = 2 * ktp2 + u
                                nc.tensor.matmul(
                                    lg[:, 512 * u:512 * (u + 1)],
                                    kt_sb[:, 128 * kt:128 * (kt + 1)],
                                    qt_sb[:, 512 * qb:512 * (qb + 1)],
                                    start=True, stop=True,
                                )
                            pt = ptp.tile([128, 1024], BF16, tag="pt")
                            nc.scalar.activation(pt[:], lg[:], EXP, scale=0.125)
                            for u in range(2):
                                kt = 2 * ktp2 + u
                                nc.tensor.matmul(
                                    vals[:],
                                    vt4[:, kt, h, :],
                                    pt[:, 512 * u:512 * (u + 1)],
                                    start=(ktp2 == 0 and u == 0),
                                    stop=(ktp2 == KT // 2 - 1 and u == 1),
                                )
                        r = 4 * h2 + qb
                        nc.vector.tensor_copy(den_flat[0:1, 512 * r:512 * (r + 1)], vals[64:65, :])
                        va = vap.tile([64, 512], BF16, tag="va")
                        nc.vector.tensor_copy(va[:], vals[0:64, :])
                        va_tiles[8 * j + r] = va

                    rec_drams = {}

                    def normalize(j):
                        # runs one pair late: the reciprocal chain has had a
                        # full pair of attention to complete, so nothing here
                        # blocks the in-order engine streams
                        rec_dram = rec_drams.pop(j)
                        for h2 in range(2):
                            for qb in range(QB):
                                r = 4 * h2 + qb
                                bcs = nrm.tile([64, 512], BF16, tag="bcs")
                                nc.sync.dma_start(
                                    bcs[:], rec_dram[r:r + 1, :].broadcast_to([64, 512])
                                )
                                nc.vector.tensor_mul(
                                    vn_all[64 * h2:64 * (h2 + 1),
                                           S * j + 512 * qb:S * j + 512 * (qb + 1)],
                                    va_tiles.pop(8 * j + r)[:],
                                    bcs[:],
                                )

                    for j in range(NPAIR):
                        kt_sb = kt_t[j % 2]
                        den_flat = nrm.tile([1, 8 * 512], BF16, tag="dflat")

                        def k_evac(tb, pps):
                            nc.vector.tensor_copy(kt_sb[:, 512 * tb:512 * (tb + 1)], pps[:])

                        def q_evac(tb, pps):
                            nc.vector.tensor_copy(
                                qt_t[j % 2][0][0:64, 512 * tb:512 * (tb + 1)], pps[0:64, :])
                            nc.vector.tensor_copy(
                                qt_t[j % 2][1][64:128, 512 * tb:512 * (tb + 1)], pps[64:128, :])

                        proj_pair(j, wk, wkp, k_evac)
                        proj_pair(j, wq, wqp, q_evac)
                        if j > 0:
                            normalize(j - 1)
                        for h2 in range(2):
                            for qb in range(QB):
                                attn_block(j, h2, qb, kt_sb, qt_t[j % 2][h2], den_flat)

                        # one cheap reciprocal for the pair's 8 denominator
                        # rows, bounced through DRAM to partition-major (engine
                        # ops cannot address partitions 1..31)
                        den_dram = dramp.tile([8, 512], BF16, tag="ddram")
                        nc.sync.dma_start(
                            den_dram.rearrange("r c -> (r c)")[None, :], den_flat[0:1, :]
                        )
                        den_sq = nrm.tile([8, 512], BF16, tag="dsq")
                        nc.sync.dma_start(den_sq[:], den_dram[:])
                        rec_sq = nrm.tile([8, 512], BF16, tag="rsq")
                        with nc.allow_low_precision(reason="denominator reciprocal in bf16"):
                            nc.vector.reciprocal(rec_sq[:], den_sq[:])
                        rec_dram = dramp.tile([8, 512], BF16, tag="rdram")
                        nc.sync.dma_start(rec_dram[:], rec_sq[:])
                        rec_drams[j] = rec_dram
                    normalize(NPAIR - 1)

                # ---- o projection tail ----
                with (
                    tc.tile_pool(name="obp", bufs=2) as obp,
                    tc.tile_pool(name="psO", bufs=2, space="PSUM") as psO,
                ):
                    vn3 = vn_all.rearrange("p (c s) -> p c s", c=NPAIR)
                    for tokb in range(S // 128):
                        o_sb = obp.tile([128, HID], F32)
                        for ob in range(2):
                            ops = psO.tile([128, 512], F32, tag="oo")
                            for c in range(NPAIR):
                                nc.tensor.matmul(
                                    ops[:],
                                    vn3[:, c, 128 * tokb:128 * (tokb + 1)],
                                    wo_sb[c][:, 512 * ob:512 * (ob + 1)],
                                    start=(c == 0), stop=(c == NPAIR - 1),
                                )
                            nc.vector.tensor_copy(o_sb[:, 512 * ob:512 * (ob + 1)], ops[:])
                        nc.sync.dma_start(o[128 * tokb:128 * (tokb + 1), :], o_sb[:])

        if n_iter > 1:
            with tc.For_i(0, n_iter, 1):
                body()
        else:
            body()

    nc.compile()
    return nc


def shard_inputs(x, w_qkv, w_o):
    x = np.asarray(x, dtype=np.float32)
    w_qkv = np.asarray(w_qkv, dtype=np.float32)
    w_o = np.asarray(w_o, dtype=np.float32)
    import ml_dtypes
    bf = ml_dtypes.bfloat16

    # w_qkv row (h*192 + c): c<64 q, 64<=c<128 k, 128<=c<192 v
    w3 = w_qkv.reshape(H, 3 * D, HID)
    wq_h = w3[:, 0:D, :]        # [H, D, HID]
    wk_h = w3[:, D:2 * D, :]
    wv_h = w3[:, 2 * D:3 * D, :]
    wo_t = w_o.T                # [HID(vals feat, h-major), HID(out)]

    cone8 = np.ones((128, 8), np.float32).astype(bf)
    in_maps = []
    for core in range(N_CORES):
        b, g = core // G, core % G
        hsel = slice(HG * g, HG * (g + 1))
        wq_g = wq_h[hsel].reshape(NPAIR, 2 * D, HID).transpose(0, 2, 1).reshape(NPAIR * HID, 128)
        wk_g = wk_h[hsel].reshape(NPAIR, 2 * D, HID).transpose(0, 2, 1).reshape(NPAIR * HID, 128)
        wv_g = wv_h[hsel].reshape(HG * D, HID).T        # [HID, 512]
        wo_g = wo_t[HG * D * g:HG * D * (g + 1), :]     # [512, HID]
        in_maps.append({
            "xt": np.ascontiguousarray(x[b].T).astype(bf),
            "wq": np.ascontiguousarray(wq_g).astype(bf),
            "wk": np.ascontiguousarray(wk_g).astype(bf),
            "wv": np.ascontiguousarray(wv_g).astype(bf),
            "wo": np.ascontiguousarray(wo_g).astype(bf),
            "cone8": cone8,
        })
    return in_maps


_NC_CACHE = {}


def get_nc(n_iter: int = 1):
    if n_iter not in _NC_CACHE:
        _NC_CACHE[n_iter] = build_nc(n_iter)
    return _NC_CACHE[n_iter]


def kernel(x, w_qkv, w_o):
    nc = get_nc(1)
    in_maps = shard_inputs(x, w_qkv, w_o)
    res = run_bass_kernel_spmd(nc, in_maps, list(range(N_CORES)))
    out = np.empty((B, S, HID), np.float32)
    for b in range(B):
        out[b] = res.results[G * b]["o"]
        for g in range(1, G):
            out[b] += res.results[G * b + g]["o"]
    return out
